# revision 12
# baseline (speedup 1.0000x reference)
"""Trainium2 Bass kernel for nn_CompositionalMLP_75763223101514.

Reference computation (per batch row b, expert k):
    xb = x.reshape(B, 16, 128)
    h  = leaky( einsum('bkm,kdm->bkd', xb, W1diag) + b1 )    # W1diag[k] = W1[k,:,k*128:(k+1)*128]
    o  = leaky( einsum('bkd,kld->bkl', h, W2) + b2 )
    out = o.reshape(B, 16*128)
with leaky(z) = z if z > 0 else 0.2 z.

Strategy: data-parallel over the batch dim across 8 NeuronCores (2048 rows
each), weights replicated.  On the host we pre-transpose each x shard to
feature-major [2048, 2048] so the contraction dim (m) lands on SBUF
partitions, extract the diagonal W1 blocks, and pre-transpose the weights
into lhsT layout.  Each core then runs, per expert k:

    MM1:  hT[d, b]  = sum_m W1T_k[m, d] * xT[k*128+m, b]     (PE, contraction 128)
    act:  h = leaky(hT + b1)  (ScalarE Prelu for the d<128 chunk + the output;
                               VectorE 2-op max(z, 0.2z) for the d>=128 chunk)
    MM2:  oT[l, b]  = sum_d W2T_k[d, l] * h[d, b]            (PE, contraction 240, accumulated)
    act:  o = leaky(oT + b2)  -> SBUF -> DMA to oT dram [k*128+l, b]

The host finally re-transposes each core's oT shard back to [2048, 2048]
batch-major and concatenates.

Matmul dtype: float32r (single-pass fp32 on the PE at full bf16 rate for
moving dim >= 256; measured max rel err ~1.4e-4 per matmul vs 2.5e-3 for
bf16).  Set DT = "bf16" to halve input DMA instead.
"""

import numpy as np
import ml_dtypes

import concourse.bacc as bacc
import concourse.mybir as mybir
from concourse.tile import TileContext
from concourse.bass_utils import run_bass_kernel_spmd

K, M, DK, L = 16, 128, 240, 128
B = 16384
NCORES = 8
BL = B // NCORES          # batch rows per core
SLOPE = 0.2
DA, DB = 128, DK - 128    # hidden split (PSUM partition limit)

DT = "fp32r"              # "bf16" | "fp32r" | "fp32"
BT = 1024                 # activation tile width (columns of local batch)

_DTYPES = {
    "bf16": (mybir.dt.bfloat16, ml_dtypes.bfloat16),
    "fp32r": (mybir.dt.float32r, np.float32),
    "fp32": (mybir.dt.float32, np.float32),
}

A = mybir.ActivationFunctionType
OP = mybir.AluOpType

_cache = {}


def _build(dt_name, repeat=1):
    """One SPMD program; all cores run it on their own batch shard."""
    dt_mm, _ = _DTYPES[dt_name]
    f32 = mybir.dt.float32
    nc = bacc.Bacc("TRN2", target_bir_lowering=False, debug=False, num_devices=NCORES)

    xT = nc.dram_tensor("xT", [K * M, BL], dt_mm, kind="ExternalInput")
    w1t = nc.dram_tensor("w1t", [K, M, DK], dt_mm, kind="ExternalInput")   # [k][m, d]
    w2t = nc.dram_tensor("w2t", [K, DK, L], dt_mm, kind="ExternalInput")   # [k][d, l]
    # bias pack: [:, k, 0]=b1[:128]  [:112, k, 1]=b1[128:]  [:, k, 2]=0.2*b1[:128]
    #            [:112, k, 3]=0.2*b1[128:]  [:, k, 4]=b2  [:, k, 5]=0.2*b2
    bias = nc.dram_tensor("bias", [128, K, 6], f32, kind="ExternalInput")
    oT = nc.dram_tensor("oT", [K * L, BL], f32, kind="ExternalOutput")

    n_half = BL // BT           # halves per expert
    n_mm = BT // NMM            # matmuls per half per chunk

    with TileContext(nc) as tc:
        with (
            tc.tile_pool(name="const", bufs=1) as cpool,
            tc.tile_pool(name="xin", bufs=XBUFS) as xpool,
            tc.tile_pool(name="h", bufs=2) as hpool,
            tc.tile_pool(name="o", bufs=OBUFS) as opool,
            tc.tile_pool(name="psum", bufs=1, space="PSUM") as psum,
        ):
            # --- resident weights/biases ---
            sw1 = cpool.tile([M, K, DK], dt_mm)
            nc.sync.dma_start(sw1[:], w1t.rearrange("k m d -> m k d"))
            sw2a = cpool.tile([DA, K, L], dt_mm)
            nc.sync.dma_start(sw2a[:], w2t[:, 0:DA, :].rearrange("k d l -> d k l"))
            sw2b = cpool.tile([DB, K, L], dt_mm)
            nc.sync.dma_start(sw2b[:], w2t[:, DA:DK, :].rearrange("k d l -> d k l"))
            sbias = cpool.tile([128, K, 6], f32)
            nc.sync.dma_start(sbias[:], bias[:])

            def bias_col(k, c, p=128):
                return sbias[0:p, k, c : c + 1]

            import contextlib
            loop_cm = tc.For_i(0, repeat, 1, hint_engines=(mybir.EngineType.PE,)) \
                if repeat > 1 else contextlib.nullcontext()
            with loop_cm:
              for k in range(K):
                  sx = xpool.tile([M, BL], dt_mm, tag="sx")
                  nc.sync.dma_start(sx[:], xT[k * M : (k + 1) * M, :])
                  so = opool.tile([L, BL], o_dt, tag="so")
                  w1a = sw1[:, k, 0:DA]
                  w1b = sw1[:, k, DA:DK]
                  w2a = sw2a[:, k, :]
                  w2b = sw2b[:, k, :]
                  for h in range(n_half):
                      hs = slice(h * BT, (h + 1) * BT)
                      pha = psum.tile([DA, BT], f32, tag="pha", bufs=1)
                      phb = psum.tile([DB, BT], f32, tag="phb", bufs=1)
                      po = psum.tile([L, BT], f32, tag="po", bufs=2)
                      for i in range(n_mm):
                          ms = slice(h * BT + i * NMM, h * BT + (i + 1) * NMM)
                          ps = slice(i * NMM, (i + 1) * NMM)
                          nc.tensor.matmul(pha[:, ps], lhsT=w1a, rhs=sx[:, ms], start=True, stop=True)
                          nc.tensor.matmul(phb[:, ps], lhsT=w1b, rhs=sx[:, ms], start=True, stop=True)
                      # leaky(z) for chunk A on ScalarE (Prelu: z>0 ? z : alpha*z)
                      sha = hpool.tile([DA, BT], dt_mm, tag="sha")
                      nc.scalar.activation(sha[:], pha[:], A.Prelu,
                                           bias=bias_col(k, 0), scale=1.0, alpha=SLOPE)
                      # leaky(z) for chunk B on VectorE: t = 0.2*psum + 0.2*b1 ; max(psum + b1, t)
                      tb = hpool.tile([DB, BT], f32, tag="tb")
                      nc.vector.tensor_scalar(tb[:], phb[:], SLOPE, bias_col(k, 3, DB),
                                              OP.mult, OP.add)
                      shb = hpool.tile([DB, BT], dt_mm, tag="shb")
                      nc.vector.scalar_tensor_tensor(shb[:], phb[:], bias_col(k, 1, DB), tb[:],
                                                     OP.add, OP.max)
                      for i in range(n_mm):
                          ps = slice(i * NMM, (i + 1) * NMM)
                          nc.tensor.matmul(po[:, ps], lhsT=w2a, rhs=sha[:, ps], start=True, stop=False)
                          nc.tensor.matmul(po[:, ps], lhsT=w2b, rhs=shb[:, ps], start=False, stop=True)
                      nc.scalar.activation(so[:, hs], po[:], A.Prelu,
                                           bias=bias_col(k, 4), scale=1.0, alpha=SLOPE)
                  nc.sync.dma_start(oT[k * L : (k + 1) * L, :], so[:])
    nc.compile()
    return nc


def _prep(x, W1, b1, W2, b2, dt_name):
    """Host-side shard + layout prep."""
    _, np_dt = _DTYPES[dt_name]
    # diagonal W1 blocks: [k, d, m] -> lhsT [k, m, d]
    W1r = W1.reshape(K, DK, K, M)
    idx = np.arange(K)
    W1d = W1r[idx, :, idx, :]                       # [k, d, m]
    w1t = np.ascontiguousarray(W1d.transpose(0, 2, 1)).astype(np_dt)   # [k, m, d]
    w2t = np.ascontiguousarray(W2.transpose(0, 2, 1)).astype(np_dt)    # [k, d, l]
    bias = np.zeros((128, K, 6), np.float32)
    bias[:, :, 0] = b1[:, 0:DA].T
    bias[0:DB, :, 1] = b1[:, DA:DK].T
    bias[:, :, 2] = SLOPE * b1[:, 0:DA].T
    bias[0:DB, :, 3] = SLOPE * b1[:, DA:DK].T
    bias[:, :, 4] = b2.T
    bias[:, :, 5] = SLOPE * b2.T

    in_maps = []
    for c in range(NCORES):
        xT = np.ascontiguousarray(x[c * BL : (c + 1) * BL, :].T).astype(np_dt)
        in_maps.append({"xT": xT, "w1t": w1t, "w2t": w2t, "bias": bias})
    return in_maps



# ---------------- Variant C: fold 0.2*W2*z into a precomputed Wc matmul ----
# leaky(z) = 0.8*relu(z) + 0.2*z, so with r = relu(W1 x + b1):
#   o_pre = W2 leaky(z) + b2 = (0.2 W2 W1) x + (0.8 W2) r + (b2 + 0.2 W2 b1)
# The Wc = 0.2*W2*W1 matmul streams straight from x (no activation dep),
# r needs only a single relu op per element, and the output activation is a
# single 2048-wide Prelu per expert.

RA_ACT_FRAC = 0.5   # fraction of rA tiles on ScalarE (rest on VectorE)
NBT = 1024          # activation tile width
XBUFS = 2
OBUFS = 2
OUT_BF16 = False
PHBUFS = 1
POBUFS = 2
NMM = 512   # matmul moving free dim (fp32/fp32r hard cap 512)


def _build_c(dt_name, repeat=1):
    dt_mm, _ = _DTYPES[dt_name]
    f32 = mybir.dt.float32
    nc = bacc.Bacc("TRN2", target_bir_lowering=False, debug=False, num_devices=NCORES)

    xT = nc.dram_tensor("xT", [K * M, BL], dt_mm, kind="ExternalInput")
    w1t = nc.dram_tensor("w1t", [K, M, DK], dt_mm, kind="ExternalInput")    # [k][m, d]
    w2r = nc.dram_tensor("w2r", [K, DK, L], dt_mm, kind="ExternalInput")    # 0.8*W2, [k][d, l]
    wc = nc.dram_tensor("wc", [K, M, L], dt_mm, kind="ExternalInput")       # 0.2*(W2@W1d).T, [k][m, l]
    bias = nc.dram_tensor("bias", [128, K, 3], f32, kind="ExternalInput")
    o_dt = mybir.dt.bfloat16 if OUT_BF16 else f32
    oT = nc.dram_tensor("oT", [K * L, BL], o_dt, kind="ExternalOutput")

    nmm = NMM if dt_name == "bf16" else min(NMM, 512)   # fp32 moving-dim limit
    n_bt = BL // NBT            # activation tiles per expert
    n_mm = NBT // nmm           # matmuls per activation tile

    with TileContext(nc) as tc:
        with (
            tc.tile_pool(name="const", bufs=1) as cpool,
            tc.tile_pool(name="xin", bufs=XBUFS) as xpool,
            tc.tile_pool(name="h", bufs=2) as hpool,
            tc.tile_pool(name="o", bufs=OBUFS) as opool,
            tc.tile_pool(name="psum", bufs=1, space="PSUM") as psum,
        ):
            sw1 = cpool.tile([M, K, DK], dt_mm)
            sw2a = cpool.tile([DA, K, L], dt_mm)
            sw2b = cpool.tile([DB, K, L], dt_mm)
            swc = cpool.tile([M, K, L], dt_mm)
            sbias = cpool.tile([128, K, 3], f32)

            def bias_col(k, c, p=128):
                return sbias[0:p, k, c : c + 1]

            import contextlib
            loop_cm = tc.For_i(0, repeat, 1, hint_engines=(mybir.EngineType.PE,)) \
                if repeat > 1 else contextlib.nullcontext()
            with loop_cm:
              ract = 0
              for k in range(K):
                sx = xpool.tile([M, BL], dt_mm, tag="sx")
                nc.sync.dma_start(sx[:], xT[k * M : (k + 1) * M, :])
                if k == 0:
                    # bulk weight loads right after x(0) so expert 0 starts fast
                    nc.sync.dma_start(sw1[:], w1t.rearrange("k m d -> m k d"))
                    nc.sync.dma_start(swc[:], wc.rearrange("k m l -> m k l"))
                    nc.sync.dma_start(sw2a[:], w2r[:, 0:DA, :].rearrange("k d l -> d k l"))
                    nc.sync.dma_start(sw2b[:], w2r[:, DA:DK, :].rearrange("k d l -> d k l"))
                    nc.sync.dma_start(sbias[:], bias[:])
                so = opool.tile([L, BL], o_dt, tag="so")
                w1a = sw1[:, k, 0:DA]
                w1b = sw1[:, k, DA:DK]
                w2a = sw2a[:, k, :]
                w2b = sw2b[:, k, :]
                wck = swc[:, k, :]
                r_dt = f32 if dt_name == "fp32" else dt_mm
                ra = hpool.tile([DA, BL], r_dt, tag="ra")
                rb = hpool.tile([DB, BL], r_dt, tag="rb")
                po_tiles = {}

                def stage1(j):
                    nonlocal ract
                    pha = psum.tile([DA, NBT], f32, tag="pha", bufs=PHBUFS)
                    phb = psum.tile([DB, NBT], f32, tag="phb", bufs=PHBUFS)
                    for i in range(n_mm):
                        ms = slice(j * NBT + i * nmm, j * NBT + (i + 1) * nmm)
                        pp = slice(i * nmm, (i + 1) * nmm)
                        nc.tensor.matmul(pha[:, pp], lhsT=w1a, rhs=sx[:, ms], start=True, stop=True)
                        nc.tensor.matmul(phb[:, pp], lhsT=w1b, rhs=sx[:, ms], start=True, stop=True)
                    ps = slice(j * NBT, (j + 1) * NBT)
                    if (ract * 977) % 1000 < RA_ACT_FRAC * 1000:
                        nc.scalar.activation(ra[:, ps], pha[:], A.Relu,
                                             bias=bias_col(k, 0), scale=1.0)
                    else:
                        nc.vector.tensor_scalar(ra[:, ps], pha[:], bias_col(k, 0), 0.0,
                                                OP.add, OP.max)
                    ract += 1
                    nc.vector.tensor_scalar(rb[:, ps], phb[:], bias_col(k, 1, DB), 0.0,
                                            OP.add, OP.max)

                def stage2(j):
                    po = psum.tile([L, NBT], f32, tag="po", bufs=POBUFS, name=f"po_{k}_{j}")
                    for i in range(n_mm):
                        ms = slice(j * NBT + i * nmm, j * NBT + (i + 1) * nmm)
                        pp = slice(i * nmm, (i + 1) * nmm)
                        nc.tensor.matmul(po[:, pp], lhsT=wck, rhs=sx[:, ms], start=True, stop=False)
                        nc.tensor.matmul(po[:, pp], lhsT=w2a, rhs=ra[:, ms], start=False, stop=False)
                        nc.tensor.matmul(po[:, pp], lhsT=w2b, rhs=rb[:, ms], start=False, stop=True)
                    ps = slice(j * NBT, (j + 1) * NBT)
                    nc.scalar.activation(so[:, ps], po[:], A.Prelu,
                                         bias=bias_col(k, 2), scale=1.0, alpha=SLOPE)

                for j in range(n_bt + 1):
                    if j < n_bt:
                        stage1(j)
                    if j >= 1:
                        stage2(j - 1)
                nc.sync.dma_start(oT[k * L : (k + 1) * L, :], so[:])
    nc.compile()
    return nc


def _prep_c(x, W1, b1, W2, b2, dt_name):
    _, np_dt = _DTYPES[dt_name]
    W1r = W1.reshape(K, DK, K, M)
    idx = np.arange(K)
    W1d = W1r[idx, :, idx, :]                                            # [k, d, m]
    w1t = np.ascontiguousarray(W1d.transpose(0, 2, 1)).astype(np_dt)     # [k, m, d]
    w2r = np.ascontiguousarray((0.8 * W2).transpose(0, 2, 1)).astype(np_dt)
    wck = 0.2 * np.matmul(W2, W1d)                                       # [k, l, m]
    wc = np.ascontiguousarray(wck.transpose(0, 2, 1)).astype(np_dt)      # [k, m, l]
    b2p = b2 + 0.2 * np.einsum("kld,kd->kl", W2, b1)
    bias = np.zeros((128, K, 3), np.float32)
    bias[:, :, 0] = b1[:, 0:DA].T
    bias[0:DB, :, 1] = b1[:, DA:DK].T
    bias[:, :, 2] = b2p.T
    in_maps = []
    for c in range(NCORES):
        xTc = np.ascontiguousarray(x[c * BL : (c + 1) * BL, :].T).astype(np_dt)
        in_maps.append({"xT": xTc, "w1t": w1t, "w2r": w2r, "wc": wc, "bias": bias})
    return in_maps


# ---------------- Variant E: variant C dataflow, bf16 end-to-end ----------
# Same math as variant C (wc = 0.2*W2@W1 folded matmul, relu-ized inner act),
# but x / weights / output all bf16 to halve DMA traffic (the fp32r baseline
# is DMA-bound: ~39 MB/pass).  Stage-1 matmuls write [*, 1024] f32 PSUM tiles
# (two 512-col matmuls each) so the relu acts run at FD=1024, amortizing the
# per-instruction PSUM access latency.  Engine budget per expert (4267 ns PE):
#   vector: ra relu x2 (FD1024) + rb relu x1  ~3.7us/expert-pair -> ~87%
#   scalar: rb relu x1 + o Prelu x4 (FD512)                      -> ~78%

def _build_e(repeat=1):
    bf16 = mybir.dt.bfloat16
    f32 = mybir.dt.float32
    nc = bacc.Bacc("TRN2", target_bir_lowering=False, debug=False, num_devices=NCORES)

    xT = nc.dram_tensor("xT", [K * M, BL], bf16, kind="ExternalInput")
    w1t = nc.dram_tensor("w1t", [M, K, DK], bf16, kind="ExternalInput")    # [m][k, d]
    w2a = nc.dram_tensor("w2a", [DA, K, L], bf16, kind="ExternalInput")    # 0.8*W2 d<128
    w2b = nc.dram_tensor("w2b", [DB, K, L], bf16, kind="ExternalInput")    # 0.8*W2 d>=128
    wc = nc.dram_tensor("wc", [M, K, L], bf16, kind="ExternalInput")       # 0.2*(W2@W1d).T
    bias = nc.dram_tensor("bias", [128, K, 3], f32, kind="ExternalInput")
    oT = nc.dram_tensor("oT", [K * L, BL], bf16, kind="ExternalOutput")

    NB1 = 1024              # stage-1 psum tile width (2 banks)
    n_t = BL // NB1         # stage-1 tiles per expert (2)
    # unroll 2 passes per For_i iteration so pass p+1's weight reload
    # (double-buffered) overlaps pass p compute instead of stalling expert 0
    # (small repeats: flat python unroll, no hardware loop — used by the sim)
    unroll = repeat if repeat <= 8 else 2
    assert repeat % unroll == 0

    with TileContext(nc) as tc:
        with (
            tc.tile_pool(name="wgt", bufs=2) as wpool,
            tc.tile_pool(name="xin", bufs=2) as xpool,
            tc.tile_pool(name="h", bufs=2) as hpool,
            tc.tile_pool(name="o", bufs=2) as opool,
            tc.tile_pool(name="psum", bufs=1, space="PSUM") as psum,
        ):
            import contextlib
            loop_cm = tc.For_i(0, repeat // unroll, 1, hint_engines=(mybir.EngineType.PE,)) \
                if repeat > unroll else contextlib.nullcontext()
            with loop_cm:
             for _u in range(unroll):
              sw1 = wpool.tile([M, K, DK], bf16, tag="sw1")
              sw2a = wpool.tile([DA, K, L], bf16, tag="sw2a")
              sw2b = wpool.tile([DB, K, L], bf16, tag="sw2b")
              swc = wpool.tile([M, K, L], bf16, tag="swc")
              sbias = wpool.tile([128, K, 3], f32, tag="sbias")

              def bias_col(k, c, p=128):
                  return sbias[0:p, k, c : c + 1]

              tix = 0   # global stage-1 tile counter (for act engine balance)
              for k in range(K):
                sx = xpool.tile([M, BL], bf16, tag="sx")
                nc.sync.dma_start(sx[:], xT[k * M : (k + 1) * M, :])
                if k == 0:
                    nc.sync.dma_start(sw1[:], w1t[:])
                    nc.sync.dma_start(swc[:], wc[:])
                    nc.sync.dma_start(sw2a[:], w2a[:])
                    nc.sync.dma_start(sw2b[:], w2b[:])
                    nc.sync.dma_start(sbias[:], bias[:])
                so = opool.tile([L, BL], bf16, tag="so")
                w1a = sw1[:, k, 0:DA]
                w1b = sw1[:, k, DA:DK]
                ra = hpool.tile([DA, BL], bf16, tag="ra")
                rb = hpool.tile([DB, BL], bf16, tag="rb")

                def stage1(j):
                    nonlocal tix
                    pha = psum.tile([DA, NB1], f32, tag="pha", bufs=2)
                    phb = psum.tile([DB, NB1], f32, tag="phb", bufs=1)
                    for i in range(NB1 // 512):
                        ms = slice(j * NB1 + i * 512, j * NB1 + (i + 1) * 512)
                        pp = slice(i * 512, (i + 1) * 512)
                        nc.tensor.matmul(pha[:, pp], lhsT=w1a, rhs=sx[:, ms], start=True, stop=True)
                        nc.tensor.matmul(phb[:, pp], lhsT=w1b, rhs=sx[:, ms], start=True, stop=True)
                    ps = slice(j * NB1, (j + 1) * NB1)
                    # ra always on vector; rb alternates vector/scalar
                    nc.vector.tensor_scalar(ra[:, ps], pha[:], bias_col(k, 0), 0.0,
                                            OP.add, OP.max)
                    if tix % 2 == 0:
                        nc.vector.tensor_scalar(rb[:, ps], phb[:], bias_col(k, 1, DB), 0.0,
                                                OP.add, OP.max)
                    else:
                        nc.scalar.activation(rb[:, ps], phb[:], A.Relu,
                                             bias=bias_col(k, 1, DB), scale=1.0)
                    tix += 1

                def stage2(j):
                    wck = swc[:, k, :]
                    w2ak = sw2a[:, k, :]
                    w2bk = sw2b[:, k, :]
                    for i in range(NB1 // 512):
                        ms = slice(j * NB1 + i * 512, j * NB1 + (i + 1) * 512)
                        po = psum.tile([L, 512], f32, tag="po", bufs=2)
                        nc.tensor.matmul(po[:], lhsT=wck, rhs=sx[:, ms], start=True, stop=False)
                        nc.tensor.matmul(po[:], lhsT=w2ak, rhs=ra[:, ms], start=False, stop=False)
                        nc.tensor.matmul(po[:], lhsT=w2bk, rhs=rb[:, ms], start=False, stop=True)
                        nc.scalar.activation(so[:, ms], po[:], A.Prelu,
                                             bias=bias_col(k, 2), scale=1.0, alpha=SLOPE)

                for j in range(n_t + 1):
                    if j < n_t:
                        stage1(j)
                    if j >= 1:
                        stage2(j - 1)
                nc.sync.dma_start(oT[k * L : (k + 1) * L, :], so[:])
    nc.compile()
    return nc


def _prep_e(x, W1, b1, W2, b2):
    bf = ml_dtypes.bfloat16
    W1r = W1.reshape(K, DK, K, M)
    idx = np.arange(K)
    W1d = W1r[idx, :, idx, :]                                            # [k, d, m]
    w1t = np.ascontiguousarray(W1d.transpose(2, 0, 1)).astype(bf)        # [m, k, d]
    w2r = 0.8 * W2                                                       # [k, l, d]
    w2a = np.ascontiguousarray(w2r[:, :, 0:DA].transpose(2, 0, 1)).astype(bf)   # [d, k, l]
    w2b = np.ascontiguousarray(w2r[:, :, DA:DK].transpose(2, 0, 1)).astype(bf)  # [d, k, l]
    wck = 0.2 * np.matmul(W2, W1d)                                       # [k, l, m]
    wc = np.ascontiguousarray(wck.transpose(2, 0, 1)).astype(bf)         # [m, k, l]
    b2p = b2 + 0.2 * np.einsum("kld,kd->kl", W2, b1)
    bias = np.zeros((128, K, 3), np.float32)
    bias[:, :, 0] = b1[:, 0:DA].T
    bias[0:DB, :, 1] = b1[:, DA:DK].T
    bias[:, :, 2] = b2p.T
    in_maps = []
    for c in range(NCORES):
        xTc = np.ascontiguousarray(x[c * BL : (c + 1) * BL, :].T).astype(bf)
        in_maps.append({"xT": xTc, "w1t": w1t, "w2a": w2a, "w2b": w2b,
                        "wc": wc, "bias": bias})
    return in_maps


# ---------------- Variant F: variant E + lag-2 software pipeline ----------
# Tile tasks (one per 1024-col stage-1 tile, 32/pass) are emitted as
# s1[t] ; s2[t-LAG], with the pending-s2 queue carried across pass
# boundaries so the For_i wraparound stays dense.  Stage-1 emits both pha
# matmuls first so the ra relu starts half a tile earlier.

LAG_F = 2

def _build_f(repeat=1):
    bf16 = mybir.dt.bfloat16
    f32 = mybir.dt.float32
    nc = bacc.Bacc("TRN2", target_bir_lowering=False, debug=False, num_devices=NCORES)

    xT = nc.dram_tensor("xT", [K * M, BL], bf16, kind="ExternalInput")
    w1t = nc.dram_tensor("w1t", [M, K, DK], bf16, kind="ExternalInput")
    w2a = nc.dram_tensor("w2a", [DA, K, L], bf16, kind="ExternalInput")
    w2b = nc.dram_tensor("w2b", [DB, K, L], bf16, kind="ExternalInput")
    wc = nc.dram_tensor("wc", [M, K, L], bf16, kind="ExternalInput")
    bias = nc.dram_tensor("bias", [128, K, 3], f32, kind="ExternalInput")
    oT = nc.dram_tensor("oT", [K * L, BL], bf16, kind="ExternalOutput")

    NB1 = 1024
    n_t = BL // NB1
    unroll = repeat if repeat <= 8 else 2
    assert repeat % unroll == 0

    from collections import deque

    with TileContext(nc) as tc:
        with (
            tc.tile_pool(name="wgt", bufs=2) as wpool,
            tc.tile_pool(name="xin", bufs=2) as xpool,
            tc.tile_pool(name="h", bufs=2) as hpool,
            tc.tile_pool(name="o", bufs=2) as opool,
            tc.tile_pool(name="psum", bufs=1, space="PSUM") as psum,
        ):
            import contextlib
            loop_cm = tc.For_i(0, repeat // unroll, 1, hint_engines=(mybir.EngineType.PE,)) \
                if repeat > unroll else contextlib.nullcontext()
            with loop_cm:
              pend = deque()   # pending (s2fn, postfn|None)
              tix = 0

              def drain(n):
                  while len(pend) > n:
                      s2fn, postfn = pend.popleft()
                      s2fn()
                      if postfn is not None:
                          postfn()

              for _u in range(unroll):
                sw1 = wpool.tile([M, K, DK], bf16, tag="sw1")
                sw2a = wpool.tile([DA, K, L], bf16, tag="sw2a")
                sw2b = wpool.tile([DB, K, L], bf16, tag="sw2b")
                swc = wpool.tile([M, K, L], bf16, tag="swc")
                sbias = wpool.tile([128, K, 3], f32, tag="sbias")
                nc.sync.dma_start(sw1[:], w1t[:])
                nc.sync.dma_start(swc[:], wc[:])
                nc.sync.dma_start(sw2a[:], w2a[:])
                nc.sync.dma_start(sw2b[:], w2b[:])
                nc.sync.dma_start(sbias[:], bias[:])

                def bias_col(k, c, p=128, sbias=sbias):
                    return sbias[0:p, k, c : c + 1]

                for k in range(K):
                    sx = xpool.tile([M, BL], bf16, tag="sx")
                    nc.sync.dma_start(sx[:], xT[k * M : (k + 1) * M, :])
                    so = opool.tile([L, BL], bf16, tag="so")
                    ra = hpool.tile([DA, BL], bf16, tag="ra")
                    rb = hpool.tile([DB, BL], bf16, tag="rb")

                    for j in range(n_t):
                        # ---- stage 1 (emitted now) ----
                        w1a = sw1[:, k, 0:DA]
                        w1b = sw1[:, k, DA:DK]
                        pha = psum.tile([DA, NB1], f32, tag="pha", bufs=2)
                        phb = psum.tile([DB, NB1], f32, tag="phb", bufs=1)
                        for i in range(NB1 // 512):
                            ms = slice(j * NB1 + i * 512, j * NB1 + (i + 1) * 512)
                            pp = slice(i * 512, (i + 1) * 512)
                            nc.tensor.matmul(pha[:, pp], lhsT=w1a, rhs=sx[:, ms], start=True, stop=True)
                        ps = slice(j * NB1, (j + 1) * NB1)
                        nc.vector.tensor_scalar(ra[:, ps], pha[:], bias_col(k, 0), 0.0,
                                                OP.add, OP.max)
                        for i in range(NB1 // 512):
                            ms = slice(j * NB1 + i * 512, j * NB1 + (i + 1) * 512)
                            pp = slice(i * 512, (i + 1) * 512)
                            nc.tensor.matmul(phb[:, pp], lhsT=w1b, rhs=sx[:, ms], start=True, stop=True)
                        if tix % 2 == 0:
                            nc.vector.tensor_scalar(rb[:, ps], phb[:], bias_col(k, 1, DB), 0.0,
                                                    OP.add, OP.max)
                        else:
                            nc.scalar.activation(rb[:, ps], phb[:], A.Relu,
                                                 bias=bias_col(k, 1, DB), scale=1.0)
                        tix += 1

                        # ---- stage 2 (deferred by LAG_F tiles) ----
                        def s2fn(k=k, j=j, sx=sx, ra=ra, rb=rb, so=so,
                                 swc=swc, sw2a=sw2a, sw2b=sw2b, bias_col=bias_col):
                            wck = swc[:, k, :]
                            w2ak = sw2a[:, k, :]
                            w2bk = sw2b[:, k, :]
                            for i in range(NB1 // 512):
                                ms = slice(j * NB1 + i * 512, j * NB1 + (i + 1) * 512)
                                po = psum.tile([L, 512], f32, tag="po", bufs=2)
                                nc.tensor.matmul(po[:], lhsT=wck, rhs=sx[:, ms], start=True, stop=False)
                                nc.tensor.matmul(po[:], lhsT=w2ak, rhs=ra[:, ms], start=False, stop=False)
                                nc.tensor.matmul(po[:], lhsT=w2bk, rhs=rb[:, ms], start=False, stop=True)
                                nc.scalar.activation(so[:, ms], po[:], A.Prelu,
                                                     bias=bias_col(k, 2), scale=1.0, alpha=SLOPE)

                        postfn = None
                        if j == n_t - 1:
                            def postfn(k=k, so=so):
                                nc.sync.dma_start(oT[k * L : (k + 1) * L, :], so[:])
                        pend.append((s2fn, postfn))
                        drain(LAG_F)
              drain(0)
    nc.compile()
    return nc


# ---------------- Variant G: variant F + b2 folded into MM2 ---------------
# rb carries a constant ones-row at partition 112; w2b gets a 113th
# contraction row holding b2' = b2 + 0.2*W2@b1.  The MM2 accumulation then
# includes the output bias for free (contraction 113 streams the same 512
# cols as 112), so the output leaky needs no bias and runs as ONE DVE op
# max(0.2*z, z).  Static act assignment per 1024-col slot:
#   DVE: ra relu (FD1024) + o half0 (FD512)   ~91%
#   ACT: rb relu (FD1024) + o half1 (FD512)   ~77%

def _build_g(repeat=1):
    bf16 = mybir.dt.bfloat16
    f32 = mybir.dt.float32
    nc = bacc.Bacc("TRN2", target_bir_lowering=False, debug=False, num_devices=NCORES)

    xT = nc.dram_tensor("xT", [K * M, BL], bf16, kind="ExternalInput")
    w1t = nc.dram_tensor("w1t", [M, K, DK], bf16, kind="ExternalInput")
    w2a = nc.dram_tensor("w2a", [DA, K, L], bf16, kind="ExternalInput")
    w2b = nc.dram_tensor("w2b", [DB + 1, K, L], bf16, kind="ExternalInput")
    wc = nc.dram_tensor("wc", [M, K, L], bf16, kind="ExternalInput")
    bias = nc.dram_tensor("bias", [128, K, 2], f32, kind="ExternalInput")
    ones = nc.dram_tensor("ones", [2, BL], bf16, kind="ExternalInput")
    oT = nc.dram_tensor("oT", [K * L, BL], bf16, kind="ExternalOutput")

    NB1 = 1024
    n_t = BL // NB1
    unroll = repeat if repeat <= 8 else 2
    assert repeat % unroll == 0

    from collections import deque

    with TileContext(nc) as tc:
        with (
            tc.tile_pool(name="wgt", bufs=2) as wpool,
            tc.tile_pool(name="xin", bufs=2) as xpool,
            tc.tile_pool(name="ha", bufs=2) as hapool,
            tc.tile_pool(name="hb", bufs=2) as hbpool,
            tc.tile_pool(name="o", bufs=2) as opool,
            tc.tile_pool(name="psum", bufs=1, space="PSUM") as psum,
        ):
            # materialize the two rb buffers and pin ones into partition 112
            rb_bufs = [hbpool.tile([DB + 1, BL], bf16, tag="rb", name=f"rb{i}")
                       for i in range(2)]
            for t in rb_bufs:
                nc.sync.dma_start(t[DB : DB + 1, :], ones[0:1, :])

            import contextlib
            loop_cm = tc.For_i(0, repeat // unroll, 1, hint_engines=(mybir.EngineType.PE,)) \
                if repeat > unroll else contextlib.nullcontext()
            with loop_cm:
              pend = deque()
              kk = 0   # global expert counter (for rb buffer alternation)

              def drain(n):
                  while len(pend) > n:
                      s2fn, postfn = pend.popleft()
                      s2fn()
                      if postfn is not None:
                          postfn()

              for _u in range(unroll):
                sw1 = wpool.tile([M, K, DK], bf16, tag="sw1")
                sw2a = wpool.tile([DA, K, L], bf16, tag="sw2a")
                sw2b = wpool.tile([DB + 1, K, L], bf16, tag="sw2b")
                swc = wpool.tile([M, K, L], bf16, tag="swc")
                sbias = wpool.tile([128, K, 2], f32, tag="sbias")
                nc.sync.dma_start(sw1[:], w1t[:])
                nc.sync.dma_start(swc[:], wc[:])
                nc.sync.dma_start(sw2a[:], w2a[:])
                nc.sync.dma_start(sw2b[:], w2b[:])
                nc.sync.dma_start(sbias[:], bias[:])

                def bias_col(k, c, p=128, sbias=sbias):
                    return sbias[0:p, k, c : c + 1]

                for k in range(K):
                    sx = xpool.tile([M, BL], bf16, tag="sx")
                    nc.sync.dma_start(sx[:], xT[k * M : (k + 1) * M, :])
                    so = opool.tile([L, BL], bf16, tag="so")
                    ra = hapool.tile([DA, BL], bf16, tag="ra")
                    rb = rb_bufs[kk % 2]
                    kk += 1

                    for j in range(n_t):
                        w1a = sw1[:, k, 0:DA]
                        w1b = sw1[:, k, DA:DK]
                        pha = psum.tile([DA, NB1], f32, tag="pha", bufs=1)
                        phb = psum.tile([DB, NB1], f32, tag="phb", bufs=1)
                        ps = slice(j * NB1, (j + 1) * NB1)
                        # phb matmuls first so the rb relus (split across both
                        # engines at FD512) free phb before the next slot
                        for i in range(NB1 // 512):
                            ms = slice(j * NB1 + i * 512, j * NB1 + (i + 1) * 512)
                            pp = slice(i * 512, (i + 1) * 512)
                            nc.tensor.matmul(phb[:, pp], lhsT=w1b, rhs=sx[:, ms], start=True, stop=True)
                        ls = slice(j * NB1, j * NB1 + 512)
                        hs = slice(j * NB1 + 512, (j + 1) * NB1)
                        nc.scalar.activation(rb[0:DB, ls], phb[:, 0:512], A.Relu,
                                             bias=bias_col(k, 1, DB), scale=1.0)
                        nc.vector.tensor_scalar(rb[0:DB, hs], phb[:, 512:NB1],
                                                bias_col(k, 1, DB), 0.0, OP.add, OP.max)
                        for i in range(NB1 // 512):
                            ms = slice(j * NB1 + i * 512, j * NB1 + (i + 1) * 512)
                            pp = slice(i * 512, (i + 1) * 512)
                            nc.tensor.matmul(pha[:, pp], lhsT=w1a, rhs=sx[:, ms], start=True, stop=True)
                        nc.vector.tensor_scalar(ra[:, ps], pha[:], bias_col(k, 0), 0.0,
                                                OP.add, OP.max)

                        def s2fn(k=k, j=j, sx=sx, ra=ra, rb=rb, so=so,
                                 swc=swc, sw2a=sw2a, sw2b=sw2b):
                            wck = swc[:, k, :]
                            w2ak = sw2a[:, k, :]
                            w2bk = sw2b[:, k, :]
                            po = psum.tile([L, NB1], f32, tag="po", bufs=2)
                            for i in range(NB1 // 512):
                                ms = slice(j * NB1 + i * 512, j * NB1 + (i + 1) * 512)
                                pp = slice(i * 512, (i + 1) * 512)
                                nc.tensor.matmul(po[:, pp], lhsT=wck, rhs=sx[:, ms], start=True, stop=False)
                                nc.tensor.matmul(po[:, pp], lhsT=w2ak, rhs=ra[:, ms], start=False, stop=False)
                                nc.tensor.matmul(po[:, pp], lhsT=w2bk, rhs=rb[:, ms], start=False, stop=True)
                            os = slice(j * NB1, (j + 1) * NB1)
                            nc.scalar.activation(so[:, os], po[:], A.Prelu,
                                                 scale=1.0, alpha=SLOPE)

                        postfn = None
                        if j == n_t - 1:
                            def postfn(k=k, so=so):
                                nc.sync.dma_start(oT[k * L : (k + 1) * L, :], so[:])
                        pend.append((s2fn, postfn))
                        drain(LAG_F)
              drain(0)
    nc.compile()
    return nc


def _prep_g(x, W1, b1, W2, b2):
    bf = ml_dtypes.bfloat16
    W1r = W1.reshape(K, DK, K, M)
    idx = np.arange(K)
    W1d = W1r[idx, :, idx, :]                                            # [k, d, m]
    w1t = np.ascontiguousarray(W1d.transpose(2, 0, 1)).astype(bf)        # [m, k, d]
    w2r = 0.8 * W2                                                       # [k, l, d]
    w2a = np.ascontiguousarray(w2r[:, :, 0:DA].transpose(2, 0, 1)).astype(bf)
    b2p = b2 + 0.2 * np.einsum("kld,kd->kl", W2, b1)                     # [k, l]
    w2b = np.empty((DB + 1, K, L), np.float32)
    w2b[0:DB] = w2r[:, :, DA:DK].transpose(2, 0, 1)
    w2b[DB] = b2p
    w2b = w2b.astype(bf)
    wck = 0.2 * np.matmul(W2, W1d)
    wc = np.ascontiguousarray(wck.transpose(2, 0, 1)).astype(bf)
    bias = np.zeros((128, K, 2), np.float32)
    bias[:, :, 0] = b1[:, 0:DA].T
    bias[0:DB, :, 1] = b1[:, DA:DK].T
    ones = np.ones((2, BL), bf)
    in_maps = []
    for c in range(NCORES):
        xTc = np.ascontiguousarray(x[c * BL : (c + 1) * BL, :].T).astype(bf)
        in_maps.append({"xT": xTc, "w1t": w1t, "w2a": w2a, "w2b": w2b,
                        "wc": wc, "bias": bias, "ones": ones})
    return in_maps


VARIANT = "g"   # "a" | "c" | "e" | "f" | "g"

_BUILDERS = {
    "a": (lambda repeat=1: _build(DT, repeat), lambda *t: _prep(*t, DT)),
    "c": (lambda repeat=1: _build_c(DT, repeat), lambda *t: _prep_c(*t, DT)),
    "e": (_build_e, _prep_e),
    "f": (_build_f, _prep_e),
    "g": (_build_g, _prep_g),
}


def _run(x, W1, b1, W2, b2, repeat=1):
    x, W1, b1, W2, b2 = (np.asarray(a) for a in (x, W1, b1, W2, b2))
    build, prep = _BUILDERS[VARIANT]
    key = (VARIANT, DT, repeat)
    if key not in _cache:
        _cache[key] = build(repeat)
    nc = _cache[key]
    in_maps = prep(x, W1, b1, W2, b2)
    res = run_bass_kernel_spmd(nc, in_maps, list(range(NCORES)))
    out = np.empty((B, K * L), np.float32)
    for c in range(NCORES):
        out[c * BL : (c + 1) * BL, :] = res.results[c]["oT"].T.astype(np.float32)
    return out, res


def kernel(x, W1, b1, W2, b2):
    out, _ = _run(x, W1, b1, W2, b2)
    return out


def measure_hw_time(x, W1, b1, W2, b2, repeat=(10000, 20000), rounds=3):
    """Estimate per-pass on-device time: the kernel body runs inside a hardware
    For_i loop; per-pass time = slope of wall-clock between two large repeat
    counts (transfer/dispatch overheads cancel; axon tunnel noise ~0.1s forces
    large R)."""
    import time as _time
    build, prep = _BUILDERS[VARIANT]
    in_maps = prep(x, W1, b1, W2, b2)
    r_lo, r_hi = repeat
    walls = {}
    for r in (r_lo, r_hi):
        key = (VARIANT, DT, r)
        if key not in _cache:
            _cache[key] = build(r)
        nc = _cache[key]
        run_bass_kernel_spmd(nc, in_maps, list(range(NCORES)))  # warm (jit compile)
        best = float("inf")
        for _ in range(rounds):
            t0 = _time.perf_counter()
            run_bass_kernel_spmd(nc, in_maps, list(range(NCORES)))
            best = min(best, _time.perf_counter() - t0)
        walls[r] = best
    hw_s = (walls[r_hi] - walls[r_lo]) / (r_hi - r_lo)
    return hw_s * 1e9, walls

